# revision 17
# baseline (speedup 1.0000x reference)
"""Trainium2 Bass kernel for GCN(2-layer) -> sum-pool -> LSTM(2-layer) -> classifier -> softmax.

Strategy:
  - Data-parallel: shard batch B=16 across 8 cores (B'=2 each). No collectives;
    host concatenates the per-core [2,10] outputs.
  - GCN scatter-add == multiply by a fixed normalized adjacency A [128,128]
    (built on host from edge_index). Per (t,b): compute (A@X)^T with
    lhsT=X chunks / rhs=A^T (layout alternation avoids all transposes), then
    @W chunks; biases folded via rank-1 ones matmuls; relu on ACT/DVE;
    node-pool via ones-column matmul. bf16 matmuls.
  - LSTM in gates-transposed orientation [128,(gate,chunk,b)], weights
    stationary; input projections hoisted to bulk matmuls; sigma(x) =
    (tanh(x/2)+1)/2 with the 1/2 pre-folded into weights so only tanh/exp/relu
    (one ACT table set) are used. State convention H=2h folded into weights.
  - Classifier + softmax on device.
"""

import numpy as np

T, B, N, F_IN = 32, 16, 128, 256
EMB, HID, NCLS = 128, 256, 10
NCORES = 8
BSH = B // NCORES  # 2
NSTEP = T
TB = T * BSH  # 64 (t,b) units per core

_CACHE = {}


def _f32(x):
    return np.ascontiguousarray(np.asarray(x, dtype=np.float32))


def _bf16(x):
    import ml_dtypes

    return np.ascontiguousarray(np.asarray(x, dtype=np.float32).astype(ml_dtypes.bfloat16))


def _host_adjacency(edge_index):
    """Dense normalized adjacency with self loops; returns A^T [N,N] fp32."""
    ei = np.asarray(edge_index, dtype=np.int64)
    loop = np.arange(N, dtype=np.int64)
    src = np.concatenate([ei[0], loop])
    dst = np.concatenate([ei[1], loop])
    deg = np.zeros((N,), np.float32)
    np.add.at(deg, dst, np.float32(1.0))
    dinv = deg.astype(np.float32) ** -0.5
    norm = (dinv[src] * dinv[dst]).astype(np.float32)
    A = np.zeros((N, N), np.float32)
    np.add.at(A, (dst, src), norm)
    return A.T.copy()


def _host_weights(inp):
    """Prepare all device weight tensors (bf16) from raw inputs."""
    W1 = _f32(inp["W1"])
    b1 = _f32(inp["b1"])
    W2 = _f32(inp["W2"])
    b2 = _f32(inp["b2"])
    Wc = _f32(inp["Wc"])
    bc = _f32(inp["bc"])

    # gate permutation (i,f,g,o) -> (i,f,o,g), and sigma-via-tanh row scaling
    perm = np.concatenate(
        [np.arange(0, 512), np.arange(768, 1024), np.arange(512, 768)]
    )
    srow = np.ones((4 * HID,), np.float32)
    srow[: 3 * HID] = 0.5  # i,f,o rows (after permutation)

    def prep_lstm(Wih, Whh, bih, bhh, in_scale):
        Wih = _f32(Wih)[perm] * srow[:, None] * in_scale
        Whh = _f32(Whh)[perm] * srow[:, None] * 0.5  # H = 2h convention
        bb = (_f32(bih) + _f32(bhh))[perm] * srow
        return Wih, Whh, bb

    Wih0p, Whh0p, b0p = prep_lstm(inp["Wih0"], inp["Whh0"], inp["bih0"], inp["bhh0"], 1.0)
    Wih1p, Whh1p, b1lp = prep_lstm(inp["Wih1"], inp["Whh1"], inp["bih1"], inp["bhh1"], 0.5)
    Wcp = Wc * 0.5

    def lhsT_chunks(Wp, kchunks):
        # Wp [4H, K]; device layout [128, kchunks*8*128]:
        # dev[:, (kc*8+jc)*128 : +128] = Wp[jc-block, kc-block].T
        M4, K = Wp.shape
        assert M4 == 4 * HID and K == kchunks * 128
        return (
            Wp.reshape(8, 128, kchunks, 128).transpose(3, 2, 0, 1).reshape(128, kchunks * 8 * 128)
        )

    dev = {
        "atw": _bf16(_host_adjacency(inp["edge_index"])),
        "w1": _bf16(W1.reshape(2, 128, F_IN).transpose(1, 0, 2).reshape(128, 2 * F_IN)),
        "b1row": _bf16(b1.reshape(1, F_IN)),
        "w2": _bf16(W2.reshape(2, 128, EMB).transpose(1, 0, 2).reshape(128, 2 * EMB)),
        "b2row": _bf16(b2.reshape(1, EMB)),
        "onesrow": _bf16(np.ones((1, 128), np.float32)),
        "onescol": _bf16(np.ones((128, 1), np.float32)),
        "wih0": _bf16(lhsT_chunks(Wih0p, 1)),
        "b0row": _bf16(b0p.reshape(1, 4 * HID)),
        "whh0": _bf16(lhsT_chunks(Whh0p, 2)),
        "wih1": _bf16(lhsT_chunks(Wih1p, 2)),
        "b1lrow": _bf16(b1lp.reshape(1, 4 * HID)),
        "whh1": _bf16(lhsT_chunks(Whh1p, 2)),
        "wc": _bf16(Wcp.reshape(4, 128, NCLS).transpose(1, 0, 2).reshape(128, 4 * NCLS)),
        "bcrow": _bf16(bc.reshape(1, NCLS)),
    }
    return dev


def _install_single_wait_legalizer():
    """This environment's walrus build supports exactly ONE sync-wait command
    per instruction (setupSyncWait 'Too many sync wait commands'). Tile freely
    emits 2+ waits. Legalize: extra waits move onto same-engine NoOps inserted
    immediately before the instruction (engines dispatch in order, so the
    blocking semantics are identical)."""
    import concourse.tile as tile
    from concourse import mybir

    if getattr(tile.TileContext, "_single_wait_patched", False):
        return

    _orig_commit = tile.TileContext._commit_instruction

    def _patched_commit(self, inst, lazy_reg_writes=True):
        si = inst.sync_info
        if (
            si is not None
            and si.on_wait
            and len(si.on_wait) > 1
            and inst.engine != mybir.EngineType.Unassigned
        ):
            waits = list(si.on_wait)
            inst.sync_info = mybir.SyncInfo(
                on_wait=[waits[-1]], on_update=list(si.on_update)
            )
            for w in waits[:-1]:
                nop = mybir.InstNoOp(
                    name=self.nc.get_next_instruction_name(),
                    engine=inst.engine,
                    sync_info=mybir.SyncInfo(on_wait=[w], on_update=[]),
                )
                self._add_instruction(nop)
        return _orig_commit(self, inst, lazy_reg_writes)

    _orig_dab = tile.TileContext._drain_and_barrier

    def _patched_dab(self, tick_clock, wait_clock):
        from concourse.vector_clock import ScopedClock

        pre = self.nc.sync.nop(nofuse=True)
        wait_clock.add_sem_waits(
            pre.ins, ScopedClock({None: tick_clock.global_clock})
        )
        si = pre.ins.sync_info
        if si is not None and si.on_wait and len(si.on_wait) > 1:
            waits = list(si.on_wait)
            pre.ins.sync_info = mybir.SyncInfo(
                on_wait=[waits[0]], on_update=list(si.on_update)
            )
            for w in waits[1:]:
                n2 = self.nc.sync.nop(nofuse=True)
                n2.ins.sync_info = mybir.SyncInfo(on_wait=[w], on_update=[])
        ret = _orig_dab(self, tick_clock, wait_clock)
        # The drain emitted by _orig_dab re-derives the same waits (the manual
        # add_sem_waits calls are stateless); they are redundant given the
        # pre-NoOp chain on the same in-order SP stream, so strip extras.
        for i in self.nc.cur_bb.bb.instructions:
            si2 = i.sync_info
            if si2 is not None and si2.on_wait and len(si2.on_wait) > 1:
                i.sync_info = mybir.SyncInfo(
                    on_wait=[si2.on_wait[0]], on_update=list(si2.on_update)
                )
        return ret

    tile.TileContext._commit_instruction = _patched_commit
    tile.TileContext._drain_and_barrier = _patched_dab
    tile.TileContext._single_wait_patched = True


def build_program():
    import concourse.bass as bass
    import concourse.tile as tile
    from concourse import mybir
    from contextlib import ExitStack

    _install_single_wait_legalizer()

    dt = mybir.dt
    AF = mybir.ActivationFunctionType
    OP = mybir.AluOpType

    nc = bass.Bass("TRN2", target_bir_lowering=False, debug=False, num_devices=NCORES)

    # ---- dram tensors ----
    x_d = nc.dram_tensor("x", [T, BSH, N, F_IN], dt.float32, kind="ExternalInput").ap()
    wname_shapes = {
        "atw": [128, 128],
        "w1": [128, 2 * F_IN],
        "b1row": [1, F_IN],
        "w2": [128, 2 * EMB],
        "b2row": [1, EMB],
        "onesrow": [1, 128],
        "onescol": [128, 1],
        "wih0": [128, 8 * 128],
        "b0row": [1, 4 * HID],
        "whh0": [128, 16 * 128],
        "wih1": [128, 16 * 128],
        "b1lrow": [1, 4 * HID],
        "whh1": [128, 16 * 128],
        "wc": [128, 4 * NCLS],
        "bcrow": [1, NCLS],
    }
    wd = {
        k: nc.dram_tensor(k, shp, dt.bfloat16, kind="ExternalInput").ap()
        for k, shp in wname_shapes.items()
    }
    out_d = nc.dram_tensor("out", [BSH, NCLS], dt.float32, kind="ExternalOutput").ap()

    with tile.TileContext(nc) as tc, ExitStack() as ctx:
        # ---- persistent sbuf: weights ----
        wpool = ctx.enter_context(tc.tile_pool(name="weights", bufs=1))
        ws = {}
        for k, shp in wname_shapes.items():
            ws[k] = wpool.tile(shp, dt.bfloat16, tag=k, name=f"w_{k}")
            nc.sync.dma_start(ws[k][:], wd[k])

        onesrow = ws["onesrow"]

        # persistent state buffers
        spool = ctx.enter_context(tc.tile_pool(name="state", bufs=1))
        y0t = spool.tile([128, 2 * NSTEP * BSH], dt.bfloat16, tag="y0t")  # H0 states
        seqT = spool.tile([128, TB], dt.bfloat16, tag="seqT")
        zh = spool.tile([128, 2 * BSH], dt.bfloat16, tag="zh")  # zero H
        zc = spool.tile([128, 2 * BSH], dt.float32, tag="zc")  # zero c2
        nc.vector.memset(zh[:], 0.0)
        nc.vector.memset(zc[:], 0.0)

        y0v = y0t[:].rearrange("p (hc t b) -> p hc t b", hc=2, t=NSTEP, b=BSH)

        # persistent psum: seq accumulator
        pseq_pool = ctx.enter_context(tc.tile_pool(name="pseq", bufs=1, space="PSUM"))
        pseq = pseq_pool.tile([128, TB], dt.float32, tag="pseq")

        # ================= GCN phase =================
        # x is fully preloaded into SBUF with fresh-buffer DMAs: the walrus
        # pseudo-DMA lowering only supports a single sync-wait command per DMA
        # instruction, so per-unit DMA-buffer cycling (which creates WAR waits
        # on DMAs) fails to compile. 8 MB of x fits in SBUF (64KB/partition).
        # NOTE: pools deliberately NOT phase-scoped either -- releasing them
        # would let LSTM-phase tiles reuse SBUF space, creating WAR deps on
        # the 8 SWDGE DMA queue semaphores (same wait-slot limit).
        xall = spool.tile([128, TB * F_IN], dt.float32, tag="xall")
        xav = xall[:].rearrange("n (t b f) -> n t b f", t=T, b=BSH, f=F_IN)
        for tch in range(16):
            nc.sync.dma_start(
                xav[:, 2 * tch : 2 * tch + 2],
                x_d[2 * tch : 2 * tch + 2].rearrange("t b n f -> n t b f"),
            )
        xallb = spool.tile([128, TB * F_IN], dt.bfloat16, tag="xallb")
        for t in range(T):
            nc.vector.tensor_copy(
                xallb[:, t * BSH * F_IN : (t + 1) * BSH * F_IN],
                xall[:, t * BSH * F_IN : (t + 1) * BSH * F_IN],
            )

        ipool = ctx.enter_context(tc.tile_pool(name="interm", bufs=3))
        pzpool = ctx.enter_context(tc.tile_pool(name="pz", bufs=2, space="PSUM"))
        pypool = ctx.enter_context(tc.tile_pool(name="py", bufs=2, space="PSUM"))
        if True:
            for t in range(T):
                for b in range(BSH):
                    tb = t * BSH + b
                    xb = xallb[:, tb * F_IN : (tb + 1) * F_IN]

                    # Zt = (A @ X)^T  [f-chunk-major 256, n 128] as [128, 2*128]
                    pz = pzpool.tile([128, 2 * N], dt.float32, tag="pz")
                    for kc in range(2):
                        # out[f,n'] = sum_n X[n,f]*A^T[n,n']  (= (A@X)^T chunk)
                        nc.tensor.matmul(
                            pz[:, kc * N : (kc + 1) * N],
                            xb[:, kc * 128 : (kc + 1) * 128],
                            ws["atw"][:],
                            start=True,
                            stop=True,
                        )
                    ztb = ipool.tile([128, 2 * N], dt.bfloat16, tag="ztb")
                    nc.vector.tensor_copy(ztb[:], pz[:])

                    # Y1 = Z @ W1 + b1   [n 128, f' 256]
                    py1 = pypool.tile([128, F_IN], dt.float32, tag="py")
                    for kc in range(2):
                        nc.tensor.matmul(
                            py1[:],
                            ztb[:, kc * 128 : (kc + 1) * 128],
                            ws["w1"][:, kc * F_IN : (kc + 1) * F_IN],
                            start=(kc == 0),
                            stop=False,
                        )
                    nc.tensor.matmul(
                        py1[:], onesrow[:, 0:128], ws["b1row"][:], start=False, stop=True
                    )
                    h1 = ipool.tile([128, F_IN], dt.bfloat16, tag="h1")
                    nc.scalar.activation(h1[:], py1[:], AF.Relu)

                    # Z2t = (A @ H1)^T
                    pz2 = pzpool.tile([128, 2 * N], dt.float32, tag="pz")
                    for kc in range(2):
                        nc.tensor.matmul(
                            pz2[:, kc * N : (kc + 1) * N],
                            h1[:, kc * 128 : (kc + 1) * 128],
                            ws["atw"][:],
                            start=True,
                            stop=True,
                        )
                    z2b = ipool.tile([128, 2 * N], dt.bfloat16, tag="z2b")
                    nc.vector.tensor_copy(z2b[:], pz2[:])

                    # Y2 = Z2 @ W2 + b2  [n 128, f2 128]
                    py2 = pypool.tile([128, EMB], dt.float32, tag="py")
                    for kc in range(2):
                        nc.tensor.matmul(
                            py2[:],
                            z2b[:, kc * 128 : (kc + 1) * 128],
                            ws["w2"][:, kc * EMB : (kc + 1) * EMB],
                            start=(kc == 0),
                            stop=False,
                        )
                    nc.tensor.matmul(
                        py2[:], onesrow[:, 0:128], ws["b2row"][:], start=False, stop=True
                    )
                    h2 = ipool.tile([128, EMB], dt.bfloat16, tag="h2")
                    nc.vector.tensor_relu(h2[:], py2[:])

                    # seq col: seq^T[:, tb] = H2^T @ ones
                    nc.tensor.matmul(
                        pseq[:, tb : tb + 1], h2[:], ws["onescol"][:], start=True, stop=True
                    )

        nc.vector.tensor_copy(seqT[:], pseq[:])

        # ================= LSTM =================
        lpool = ctx.enter_context(tc.tile_pool(name="lstm", bufs=3))
        pg_pool = ctx.enter_context(tc.tile_pool(name="pgates", bufs=2, space="PSUM"))

        def lstm_layer(wih_key, nkc_in, rhs_in_fn, bias_key, whh_key, h_out_fn, c_tag):
            """Generic LSTM layer. rhs_in_fn(kc)->AP [128, TB] bulk input;
            h_out_fn(t)->(write AP [128,2,BSH], read_fn(kc,t)->AP [128,BSH])."""
            pg = pg_pool.tile([128, 8 * NSTEP * BSH], dt.float32, tag="pg")
            pgv = pg[:].rearrange("p (j t b) -> p j t b", j=8, t=NSTEP, b=BSH)
            # bulk input projection + bias.
            # NOTE psum semantics: start=True clears has_written for the WHOLE
            # bank, so exactly one start=True per pg tile (first MM); everything
            # else accumulates / first-writes per element.
            for jc in range(8):
                for kc in range(nkc_in):
                    nc.tensor.matmul(
                        pgv[:, jc],
                        ws[wih_key][:, (kc * 8 + jc) * 128 : (kc * 8 + jc + 1) * 128],
                        rhs_in_fn(kc),
                        start=(jc == 0 and kc == 0),
                        stop=False,
                        skip_group_check=True,
                    )
                nc.tensor.matmul(
                    pgv[:, jc],
                    ws[bias_key][:, jc * 128 : (jc + 1) * 128],
                    onesrow[:, 0:TB],
                    start=False,
                    stop=False,
                    skip_group_check=True,
                )
            c_prev = zc
            h_read = None
            for t in range(NSTEP):
                for jc in range(8):
                    for kc in range(2):
                        rhs = (
                            zh[:, kc * BSH : (kc + 1) * BSH]
                            if t == 0
                            else h_read(kc, t - 1)
                        )
                        nc.tensor.matmul(
                            pgv[:, jc, t],
                            ws[whh_key][:, (kc * 8 + jc) * 128 : (kc * 8 + jc + 1) * 128],
                            rhs,
                            start=False,
                            stop=(t == NSTEP - 1 and jc == 7 and kc == 1),
                            skip_group_check=True,
                        )
                tt = lpool.tile([128, 8 * BSH], dt.float32, tag="tt")
                nc.scalar.activation(
                    tt[:].rearrange("p (j b) -> p j b", j=8, b=BSH),
                    pgv[:, :, t, :],
                    AF.Tanh,
                )
                ti = tt[:, 0 * BSH : 2 * BSH]
                tf = tt[:, 2 * BSH : 4 * BSH]
                to = tt[:, 4 * BSH : 6 * BSH]
                tg = tt[:, 6 * BSH : 8 * BSH]
                u = lpool.tile([128, 2 * BSH], dt.float32, tag="u")
                nc.vector.scalar_tensor_tensor(u[:], ti, 1.0, tg, OP.add, OP.mult)
                v = lpool.tile([128, 2 * BSH], dt.float32, tag="v")
                nc.vector.scalar_tensor_tensor(v[:], tf, 1.0, c_prev[:], OP.add, OP.mult)
                c_new = lpool.tile([128, 2 * BSH], dt.float32, tag=c_tag)
                nc.vector.scalar_tensor_tensor(c_new[:], v[:], 0.5, u[:], OP.mult, OP.add)
                tc_ = lpool.tile([128, 2 * BSH], dt.float32, tag="tc")
                nc.scalar.activation(tc_[:], c_new[:], AF.Tanh, scale=0.5)
                h_write, h_read = h_out_fn(t)
                nc.vector.scalar_tensor_tensor(
                    h_write,
                    to.rearrange("p (hc b) -> p hc b", hc=2, b=BSH),
                    1.0,
                    tc_[:].rearrange("p (hc b) -> p hc b", hc=2, b=BSH),
                    OP.add,
                    OP.mult,
                )
                c_prev = c_new
            return h_read

        # layer 0: states into y0t buffer
        def h0_out_fn(t):
            def read(kc, tt_):
                return y0v[:, kc, tt_, :]

            return y0v[:, :, t, :], read

        h0_read = lstm_layer("wih0", 1, lambda kc: seqT[:], "b0row", "whh0", h0_out_fn, "c0")

        # layer 1: states into per-step tiles
        h1_tiles = {}

        def h1_out_fn(t):
            htile = lpool.tile([128, 2 * BSH], dt.bfloat16, tag="h1l")
            h1_tiles[t] = htile

            def read(kc, tt_):
                return h1_tiles[tt_][:, kc * BSH : (kc + 1) * BSH]

            return htile[:].rearrange("p (hc b) -> p hc b", hc=2, b=BSH), read

        h1_read = lstm_layer(
            "wih1", 2, lambda kc: y0v[:, kc], "b1lrow", "whh1", h1_out_fn, "c1"
        )

        # ================= classifier + softmax =================
        cpool = ctx.enter_context(tc.tile_pool(name="cls", bufs=1))
        pc_pool = ctx.enter_context(tc.tile_pool(name="pcls", bufs=1, space="PSUM"))
        r0 = cpool.tile([128, 2 * BSH], dt.bfloat16, tag="r0")
        r1 = cpool.tile([128, 2 * BSH], dt.bfloat16, tag="r1")
        nc.scalar.activation(
            r0[:].rearrange("p (hc b) -> p hc b", hc=2, b=BSH), y0v[:, :, NSTEP - 1, :], AF.Relu
        )
        nc.scalar.activation(r1[:], h1_tiles[NSTEP - 1][:], AF.Relu)
        pl = pc_pool.tile([BSH, NCLS], dt.float32, tag="pl")
        for i, rt in enumerate([r0, r1]):
            for hc in range(2):
                nc.tensor.matmul(
                    pl[:],
                    rt[:, hc * BSH : (hc + 1) * BSH],
                    ws["wc"][:, (2 * i + hc) * NCLS : (2 * i + hc + 1) * NCLS],
                    start=(i == 0 and hc == 0),
                    stop=False,
                )
        nc.tensor.matmul(pl[:], onesrow[:, 0:BSH], ws["bcrow"][:], start=False, stop=True)

        ee = cpool.tile([BSH, NCLS], dt.float32, tag="ee")
        ssum = cpool.tile([BSH, 1], dt.float32, tag="ssum")
        nc.scalar.activation(ee[:], pl[:], AF.Exp, accum_out=ssum[:])
        rr = cpool.tile([BSH, 1], dt.float32, tag="rr")
        nc.vector.reciprocal(rr[:], ssum[:])
        oo = cpool.tile([BSH, NCLS], dt.float32, tag="oo")
        nc.vector.tensor_scalar_mul(oo[:], ee[:], rr[:])
        nc.sync.dma_start(out_d, oo[:])

    return nc


def _get_program():
    if "nc" not in _CACHE:
        _CACHE["nc"] = build_program()
    return _CACHE["nc"]


def kernel(**inputs):
    from concourse.bass_utils import run_bass_kernel_spmd

    nc = _get_program()
    dev = _host_weights(inputs)
    x = _f32(inputs["node_features"])
    in_maps = []
    for c in range(NCORES):
        m = dict(dev)
        m["x"] = np.ascontiguousarray(x[:, c * BSH : (c + 1) * BSH])
        in_maps.append(m)
    res = run_bass_kernel_spmd(nc, in_maps, list(range(NCORES)))
    out = np.concatenate([res.results[c]["out"] for c in range(NCORES)], axis=0)
    return out.astype(np.float32)


if __name__ == "__main__":
    import jax

    jax.config.update("jax_platforms", "cpu")  # not used; placeholder
